# revision 1
# baseline (speedup 1.0000x reference)
"""CARAFE upsampling kernel for 8 Trainium2 NeuronCores.

Reference op (per batch b):
  xc   = conv1x1(x, w1) + b1                     # (CC=64, H, W)
  mask = conv3x3(xc, w2, pad=1) + b2             # (100, H, W)
  mask = softmax over the 25 kernel taps (per q in 4 = SF*SF groups)
  out[q, c, h, w] = sum_k mask[q, k, h, w] * x[c, h+di-2, w+dj-2]
  out pixel-shuffled by SF=2 -> (C, 2H, 2W)

Sharding: 8 shards = batch(4) x H-halves(2). Each core gets a padded
x slice [256, 36, 68] (2 halo rows / 2 zero-pad cols on each side) and
produces out rows [32 rows x 64 cols x 4 quadrants x 256 ch]; the host
performs the pixel shuffle + concat.
"""

import os
from functools import lru_cache

import numpy as np

import concourse.bass as bass
import concourse.mybir as mybir
from concourse import bacc
import concourse.tile as tile
from concourse.bass_utils import run_bass_kernel_spmd

F32 = mybir.dt.float32
BF16 = mybir.dt.bfloat16
import ml_dtypes as _mld

_BF16NP = _mld.bfloat16

# Problem constants (hardcoded; kernel.py must be self-contained).
B, C, H, W = 4, 256, 64, 64
CC = 64           # compressed channels
SF = 2            # scale factor
K5 = 5            # up-kernel
KA = K5 * K5      # 25 taps
NQ = SF * SF      # 4 quadrants
NM = NQ * KA      # 100 mask channels

HL = 32           # local (per-shard) output rows
HP = HL + 4       # padded rows
WP = W + 4        # padded cols
NPIX = HL * W     # 2048 output pixels per shard
NPADPIX = HP * WP # 2448 padded pixels

N_CORES = 8


def _build_program(trace_debug: bool = False):
    """Build the SPMD Bass program (identical on all cores)."""
    nc = bacc.Bacc("TRN2", target_bir_lowering=False, debug=False)

    # ---- DRAM parameters -------------------------------------------------
    x0_d = nc.dram_tensor("x0", [128, HP, WP], F32, kind="ExternalInput")
    x1_d = nc.dram_tensor("x1", [128, HP, WP], F32, kind="ExternalInput")
    w1t_d = nc.dram_tensor("w1t", [2, 128, CC], F32, kind="ExternalInput")
    w2t_d = nc.dram_tensor("w2t", [CC, 9, NM], F32, kind="ExternalInput")
    b1_d = nc.dram_tensor("b1v", [CC, 1], F32, kind="ExternalInput")
    b2_d = nc.dram_tensor("b2v", [NM, 1], F32, kind="ExternalInput")
    osum_d = nc.dram_tensor("osum", [NM, NQ], F32, kind="ExternalInput")
    orep_d = nc.dram_tensor("orep", [NQ, NM], F32, kind="ExternalInput")
    # gather selectors: sel4[k] is [NM, NQ] with column q = one-hot(q*25+k)
    sel4_d = nc.dram_tensor("sel4", [NM, KA, NQ], F32, kind="ExternalInput")
    # broadcast selectors: selb[q] is [NQ, 128] row-q of ones
    selb_d = nc.dram_tensor("selb", [NQ, NQ, 128], F32, kind="ExternalInput")
    # bf16 copies of the padded input, even- and odd-shifted (for DVE 2x mode
    # alignment: a window starting at odd dj reads the odd-shifted copy at an
    # even element offset)
    xbe_d = [nc.dram_tensor(f"xbe{c}", [128, HP, WP], BF16, kind="ExternalInput")
             for c in range(2)]
    xbo_d = [nc.dram_tensor(f"xbo{c}", [128, HP, WP], BF16, kind="ExternalInput")
             for c in range(2)]

    out_d = nc.dram_tensor("out", [2, 128, NQ, NPIX], F32, kind="ExternalOutput")
    msk_dbg_d = None
    if trace_debug:
        msk_dbg_d = nc.dram_tensor("msk_dbg", [NM, NPIX], F32, kind="ExternalOutput")

    with tile.TileContext(nc) as tc:
        with (
            tc.tile_pool(name="xpool", bufs=1) as xpool,
            tc.tile_pool(name="wpool", bufs=1) as wpool,
            tc.tile_pool(name="mpool", bufs=1) as mpool,
            tc.tile_pool(name="acc", bufs=1) as accpool,
            tc.tile_pool(name="scratch", bufs=2) as scratch,
            tc.tile_pool(name="psum", bufs=2, space="PSUM") as psum,
            tc.tile_pool(name="psum_rep", bufs=4, space="PSUM") as psum_rep,
        ):
            # ---- load inputs -------------------------------------------
            x0 = xpool.tile([128, HP, WP], F32)
            x1 = xpool.tile([128, HP, WP], F32)
            nc.sync.dma_start(x0[:], x0_d[:])
            nc.sync.dma_start(x1[:], x1_d[:])
            xbe0 = xpool.tile([128, HP, WP], BF16, tag="xbe0")
            xbe1 = xpool.tile([128, HP, WP], BF16, tag="xbe1")
            xbo0 = xpool.tile([128, HP, WP], BF16, tag="xbo0")
            xbo1 = xpool.tile([128, HP, WP], BF16, tag="xbo1")
            nc.sync.dma_start(xbe0[:], xbe_d[0][:])
            nc.sync.dma_start(xbe1[:], xbe_d[1][:])
            nc.sync.dma_start(xbo0[:], xbo_d[0][:])
            nc.sync.dma_start(xbo1[:], xbo_d[1][:])

            # partition dim must come first for SBUF: store as [128, 2, CC]
            w1sb = wpool.tile([128, 2, CC], F32, tag="w1sb")
            nc.sync.dma_start(w1sb[:, 0, :], w1t_d[0])
            nc.sync.dma_start(w1sb[:, 1, :], w1t_d[1])

            w2sb = wpool.tile([CC, 9, NM], F32, tag="w2sb")
            nc.sync.dma_start(w2sb[:], w2t_d[:])

            b1c = wpool.tile([CC, 1], F32, tag="b1c")
            nc.sync.dma_start(b1c[:], b1_d[:])
            b2c = wpool.tile([NM, 1], F32, tag="b2c")
            nc.sync.dma_start(b2c[:], b2_d[:])
            osum = wpool.tile([NM, NQ], F32, tag="osum")
            nc.sync.dma_start(osum[:], osum_d[:])
            orep = wpool.tile([NQ, NM], F32, tag="orep")
            nc.sync.dma_start(orep[:], orep_d[:])
            sel4 = wpool.tile([NM, KA, NQ], F32, tag="sel4")
            nc.sync.dma_start(sel4[:], sel4_d[:])
            selb = wpool.tile([NQ, NQ, 128], F32, tag="selb")
            nc.sync.dma_start(selb[:], selb_d[:])

            # ---- PE fences: make PE observe each input-DMA semaphore on a
            # tiny standalone matmul, so real (accumulating) matmuls don't
            # exceed the per-instruction sync-wait limit.
            for fap in (
                x0[:, 0, 0:1], x1[:, 0, 0:1], w1sb[:, 0, 0:1],
                w2sb[:, 0, 0:1], osum[:, 0:1], orep[:, 0:1],
                sel4[:, 0, 0:1], selb[:, 0, 0:1],
            ):
                psf = psum.tile([1, 1], F32, tag="psf")
                nc.tensor.matmul(psf[:], fap, fap, start=True, stop=True)

            # ---- stage A: conv1x1  xc[cc, pix'] over the padded grid ----
            xc = mpool.tile([CC, HP, WP], F32, tag="xc")
            xc_flat = xc[:].rearrange("c h w -> c (h w)")
            x0_flat = x0[:].rearrange("c h w -> c (h w)")
            x1_flat = x1[:].rearrange("c h w -> c (h w)")
            CHUNK = 512
            nchunks = (NPADPIX + CHUNK - 1) // CHUNK  # 5 (last = 400)
            for i in range(nchunks):
                n0 = i * CHUNK
                n1 = min(NPADPIX, n0 + CHUNK)
                ps = psum.tile([CC, CHUNK], F32, tag="ps")
                nc.tensor.matmul(
                    ps[:, : n1 - n0], w1sb[:, 0, :], x0_flat[:, n0:n1],
                    start=True, stop=False,
                )
                nc.tensor.matmul(
                    ps[:, : n1 - n0], w1sb[:, 1, :], x1_flat[:, n0:n1],
                    start=False, stop=True,
                )
                # += b1 while copying PSUM -> SBUF
                nc.vector.tensor_scalar_add(
                    xc_flat[:, n0:n1], ps[:, : n1 - n0], b1c[:, 0:1]
                )

            # ---- stage B: conv3x3 -> mask_raw, fused exp((.)+b2) -------
            # output pixels: h in 0..31 (padded row h+2), w in 0..63 (padded col w+2)
            msk_e = mpool.tile([NM, NPIX], F32, tag="msk_e")  # exp(mask_raw)
            HROWS = 8  # rows per 512-chunk
            for i in range(HL // HROWS):  # 4 chunks
                psm = psum.tile([NM, HROWS, W], F32, tag="ps")
                for tap in range(9):
                    dy, dx = tap // 3, tap % 3
                    rhs = xc[:, i * HROWS + 1 + dy : i * HROWS + 1 + dy + HROWS,
                             1 + dx : 1 + dx + W]
                    nc.tensor.matmul(
                        psm[:], w2sb[:, tap, :], rhs,
                        start=(tap == 0), stop=(tap == 8),
                    )
                me = msk_e[:].rearrange("m (h w) -> m h w", w=W)
                nc.scalar.activation(
                    me[:, i * HROWS : (i + 1) * HROWS, :], psm[:],
                    mybir.ActivationFunctionType.Exp, bias=b2c[:, 0:1],
                )

            # ---- stage C: softmax denominators + normalize -------------
            rs = mpool.tile([NQ, NPIX], F32, tag="rs")  # 1/sum per (q, pix)
            for i in range(NPIX // CHUNK):  # 4
                pss = psum.tile([NQ, CHUNK], F32, tag="ps")
                nc.tensor.matmul(
                    pss[:], osum[:], msk_e[:, i * CHUNK : (i + 1) * CHUNK],
                    start=True, stop=True,
                )
                nc.vector.reciprocal(rs[:, i * CHUNK : (i + 1) * CHUNK], pss[:])

            msk_n = mpool.tile([NM, NPIX], F32, tag="msk_n")
            for i in range(NPIX // CHUNK):
                psr = psum.tile([NM, CHUNK], F32, tag="ps")
                nc.tensor.matmul(
                    psr[:], orep[:], rs[:, i * CHUNK : (i + 1) * CHUNK],
                    start=True, stop=True,
                )
                nc.vector.tensor_mul(
                    msk_n[:, i * CHUNK : (i + 1) * CHUNK],
                    msk_e[:, i * CHUNK : (i + 1) * CHUNK], psr[:],
                )

            if trace_debug:
                nc.sync.dma_start(msk_dbg_d[:], msk_n[:])

            # ---- stage D1: combine (correctness-first) -----------------
            # acc[ch][c, q, pix] += msk_n[q*25+k, pix] * x[ch][c, window_k]
            acc0 = accpool.tile([128, NQ, NPIX], F32, tag="acc0")
            acc1 = accpool.tile([128, NQ, NPIX], F32, tag="acc1")
            nc.vector.memset(acc0[:], 0.0)
            nc.gpsimd.memset(acc1[:], 0.0)

            nadds = 0
            xbe = (xbe0, xbe1)
            xbo = (xbo0, xbo1)
            accs = (acc0, acc1)
            for k in range(KA):
                di, dj = k // 5, k % 5
                # pick the x copy whose window start is 4B-aligned in bf16
                xw, djw = (xbe, dj) if dj % 2 == 0 else (xbo, dj - 1)
                # stage 1: gather the 4 q-rows of tap k to partitions 0..3
                m4 = scratch.tile([NQ, NPIX], F32, tag="m4")
                for i in range(NPIX // CHUNK):
                    p4 = psum.tile([NQ, CHUNK], F32, tag="ps")
                    nc.tensor.matmul(
                        p4[:], sel4[:, k, :],
                        msk_n[:, i * CHUNK : (i + 1) * CHUNK],
                        start=True, stop=True,
                    )
                    nc.scalar.copy(m4[:, i * CHUNK : (i + 1) * CHUNK], p4[:])
                for q in range(NQ):
                    # stage 2: broadcast row q of m4 across 128 partitions
                    # (PE), cast to bf16 (ACT), multiply vs x-window (DVE
                    # 2x bf16), accumulate into fp32 acc (DVE/GPSIMD).
                    prod0 = scratch.tile([128, HL, W], BF16, tag="prod0")
                    prod1 = scratch.tile([128, HL, W], BF16, tag="prod1")
                    prods = [prod0, prod1]
                    prepb = scratch.tile([128, NPIX], BF16, tag="prepb")
                    for i in range(NPIX // CHUNK):
                        prep = psum_rep.tile([128, CHUNK], F32, tag="prep")
                        nc.tensor.matmul(
                            prep[:],
                            selb[:, q, :],
                            m4[:, i * CHUNK : (i + 1) * CHUNK],
                            start=True, stop=True,
                        )
                        nc.scalar.copy(
                            prepb[:, i * CHUNK : (i + 1) * CHUNK], prep[:]
                        )
                    prepv = prepb[:].rearrange("c (h w) -> c h w", w=W)
                    for ch in range(2):
                        xwin = xw[ch][:, di : di + HL, djw : djw + W]
                        nc.vector.tensor_mul(prods[ch][:], xwin, prepv)
                    for ch in range(2):
                        accv = accs[ch][:].rearrange("c q (h w) -> c q h w", w=W)
                        # split the adds between DVE and GPSIMD (~2:1)
                        eng = nc.gpsimd if (nadds % 2 == 0) else nc.vector
                        nadds += 1
                        eng.tensor_add(accv[:, q], accv[:, q], prods[ch][:])

            # ---- write out ---------------------------------------------
            nc.sync.dma_start(out_d[0], acc0[:])
            nc.sync.dma_start(out_d[1], acc1[:])

    nc.compile()
    return nc


@lru_cache(maxsize=2)
def _get_program(trace_debug: bool = False):
    return _build_program(trace_debug)


def _host_prep(x, w1, b1, w2, b2):
    """Build per-core input maps."""
    x = np.asarray(x, np.float32)
    w1 = np.asarray(w1, np.float32)
    b1 = np.asarray(b1, np.float32).reshape(CC, 1)
    w2 = np.asarray(w2, np.float32)
    b2 = np.asarray(b2, np.float32).reshape(NM, 1)

    w1t = np.ascontiguousarray(
        w1[:, :, 0, 0].T.reshape(2, 128, CC)
    )  # [c-tile, 128, CC]
    # w2: (100, 64, 3, 3) -> [cc, tap, m]
    w2t = np.ascontiguousarray(w2.transpose(1, 2, 3, 0).reshape(CC, 9, NM))
    osum = np.zeros((NM, NQ), np.float32)
    for q in range(NQ):
        osum[q * KA : (q + 1) * KA, q] = 1.0
    orep = np.ascontiguousarray(osum.T)
    sel4 = np.zeros((NM, KA, NQ), np.float32)
    for k in range(KA):
        for q in range(NQ):
            sel4[q * KA + k, k, q] = 1.0
    selb = np.zeros((NQ, NQ, 128), np.float32)
    for q in range(NQ):
        selb[q, q, :] = 1.0

    in_maps = []
    for s in range(N_CORES):
        b, hh = s // 2, s % 2
        h0 = hh * HL
        xpad = np.zeros((C, HP, WP), np.float32)
        r0 = max(0, h0 - 2)
        r1 = min(H, h0 + HL + 2)
        xpad[:, (r0 - h0 + 2) : (r1 - h0 + 2), 2 : 2 + W] = x[b, :, r0:r1, :]
        xb = xpad.astype(_BF16NP)
        xbo = np.zeros_like(xb)
        xbo[:, :, :-1] = xb[:, :, 1:]
        in_maps.append(
            {
                "x0": np.ascontiguousarray(xpad[:128]),
                "x1": np.ascontiguousarray(xpad[128:]),
                "xbe0": np.ascontiguousarray(xb[:128]),
                "xbe1": np.ascontiguousarray(xb[128:]),
                "xbo0": np.ascontiguousarray(xbo[:128]),
                "xbo1": np.ascontiguousarray(xbo[128:]),
                "w1t": w1t,
                "w2t": w2t,
                "b1v": b1,
                "b2v": b2,
                "osum": osum,
                "orep": orep,
                "sel4": sel4,
                "selb": selb,
            }
        )
    return in_maps


def _host_post(results):
    """Reassemble full output from per-core results."""
    out = np.empty((B, C, H * SF, W * SF), np.float32)
    for s in range(N_CORES):
        b, hh = s // 2, s % 2
        o = results[s]["out"]  # [2, 128, NQ, NPIX]
        o = o.reshape(2, 128, NQ, HL, W).reshape(C, SF, SF, HL, W)
        # out[c, 2h+sh, 2w+sw] = o[c, sh, sw, h, w]
        o = o.transpose(0, 3, 1, 4, 2).reshape(C, HL * SF, W * SF)
        out[b, :, hh * HL * SF : (hh + 1) * HL * SF, :] = o
    return out


def kernel(x, w1, b1, w2, b2):
    nc = _get_program(bool(int(os.environ.get("CARAFE_DEBUG", "0"))))
    in_maps = _host_prep(x, w1, b1, w2, b2)
    res = run_bass_kernel_spmd(nc, in_maps, list(range(N_CORES)))
    return _host_post(res.results)



# revision 5
# speedup vs baseline: 1.2671x; 1.2671x over previous
"""CARAFE upsampling kernel for 8 Trainium2 NeuronCores.

Reference op (per batch b):
  xc   = conv1x1(x, w1) + b1                     # (CC=64, H, W)
  mask = conv3x3(xc, w2, pad=1) + b2             # (100, H, W)
  mask = softmax over the 25 kernel taps (per q in 4 = SF*SF groups)
  out[q, c, h, w] = sum_k mask[q, k, h, w] * x[c, h+di-2, w+dj-2]
  out pixel-shuffled by SF=2 -> (C, 2H, 2W)

Sharding: 8 shards = batch(4) x H-halves(2).

Device-side layout: the mask pipeline (stages A-C) runs in channel-major
layout; the combine runs in PIXEL-major layout (pixels on partitions,
channels on the free dim) so each mask value is a per-partition scalar
and the 25-tap accumulation is a chain of fused scalar_tensor_tensor
MACs split across the DVE and GPSIMD engines. The host ships pre-shifted
bf16 x-window tiles xtw[(rr,w)=128, 35 row-pair starts, 5 dj, 256 c].
"""

import os
from functools import lru_cache

import numpy as np

import concourse.bass as bass
import concourse.mybir as mybir
from concourse import bacc
import concourse.tile as tile
from concourse.bass_utils import run_bass_kernel_spmd

F32 = mybir.dt.float32
BF16 = mybir.dt.bfloat16
import ml_dtypes as _mld

_BF16NP = _mld.bfloat16

# Problem constants (hardcoded; kernel.py must be self-contained).
B, C, H, W = 4, 256, 64, 64
CC = 64           # compressed channels
SF = 2            # scale factor
K5 = 5            # up-kernel
KA = K5 * K5      # 25 taps
NQ = SF * SF      # 4 quadrants
NM = NQ * KA      # 100 mask channels

HL = 32           # local (per-shard) output rows
HP = HL + 4       # padded rows
WP = W + 4        # padded cols
NPIX = HL * W     # 2048 output pixels per shard
NPADPIX = HP * WP # 2448 padded pixels
NBLK = HL // 2    # 16 pixel blocks of 128 = 2 rows x 64 cols
NRP = HL + 3      # 35 row-pair starts (padded rows rp, rp+1 for rp in 0..34)

N_CORES = 8

# Combine-chain engine split: DVE op ~329 ns vs GPSIMD op ~450 ns.
_DVE_NS, _GPS_NS = 329.0, 450.0


def _build_program():
    """Build the SPMD Bass program (identical on all cores)."""
    nc = bacc.Bacc("TRN2", target_bir_lowering=False, debug=False)

    # ---- DRAM parameters -------------------------------------------------
    x0_d = nc.dram_tensor("x0", [128, HP, WP], F32, kind="ExternalInput")
    x1_d = nc.dram_tensor("x1", [128, HP, WP], F32, kind="ExternalInput")
    w1t_d = nc.dram_tensor("w1t", [2, 128, CC], F32, kind="ExternalInput")
    w2t_d = nc.dram_tensor("w2t", [CC, 9, NM], F32, kind="ExternalInput")
    b1_d = nc.dram_tensor("b1v", [CC, 1], F32, kind="ExternalInput")
    b2_d = nc.dram_tensor("b2v", [NM, 1], F32, kind="ExternalInput")
    osum_d = nc.dram_tensor("osum", [NM, NQ], F32, kind="ExternalInput")
    orep_d = nc.dram_tensor("orep", [NQ, NM], F32, kind="ExternalInput")
    i100_d = nc.dram_tensor("i100", [NM, NM], F32, kind="ExternalInput")
    # pre-shifted pixel-major window tiles:
    # xtw[(rr*64+w), rp, dj, c] = xpadT[rp+rr, w+dj, c]  (bf16)
    xtw_d = nc.dram_tensor("xtw", [128, NRP, K5, C], BF16, kind="ExternalInput")

    # out[blk, q, (rr*64+w), c]
    out_d = nc.dram_tensor("out", [NBLK, NQ, 128, C], F32, kind="ExternalOutput")

    with tile.TileContext(nc) as tc:
        with (
            tc.tile_pool(name="xpool", bufs=1) as xpool,
            tc.tile_pool(name="wpool", bufs=1) as wpool,
            tc.tile_pool(name="mpool", bufs=1) as mpool,
            tc.tile_pool(name="acc", bufs=8) as accpool,
            tc.tile_pool(name="scratch", bufs=4) as scratch,
            tc.tile_pool(name="psum", bufs=2, space="PSUM") as psum,
            tc.tile_pool(name="psum_t", bufs=4, space="PSUM") as psum_t,
        ):
            # ---- load inputs -------------------------------------------
            x0 = xpool.tile([128, HP, WP], F32)
            x1 = xpool.tile([128, HP, WP], F32)
            nc.sync.dma_start(x0[:], x0_d[:])
            nc.sync.dma_start(x1[:], x1_d[:])

            xtw = xpool.tile([128, NRP, K5, C], BF16, tag="xtw")
            # split by rp range so early blocks' windows land first
            RSPLIT = (0, 9, 18, 27, NRP)
            for s in range(4):
                nc.sync.dma_start(
                    xtw[:, RSPLIT[s] : RSPLIT[s + 1]],
                    xtw_d[:, RSPLIT[s] : RSPLIT[s + 1]],
                )

            # partition dim must come first for SBUF: store as [128, 2, CC]
            w1sb = wpool.tile([128, 2, CC], F32, tag="w1sb")
            nc.sync.dma_start(w1sb[:, 0, :], w1t_d[0])
            nc.sync.dma_start(w1sb[:, 1, :], w1t_d[1])

            w2sb = wpool.tile([CC, 9, NM], F32, tag="w2sb")
            nc.sync.dma_start(w2sb[:], w2t_d[:])

            b1c = wpool.tile([CC, 1], F32, tag="b1c")
            nc.sync.dma_start(b1c[:], b1_d[:])
            b2c = wpool.tile([NM, 1], F32, tag="b2c")
            nc.sync.dma_start(b2c[:], b2_d[:])
            osum = wpool.tile([NM, NQ], F32, tag="osum")
            nc.sync.dma_start(osum[:], osum_d[:])
            orep = wpool.tile([NQ, NM], F32, tag="orep")
            nc.sync.dma_start(orep[:], orep_d[:])
            i100 = wpool.tile([NM, NM], F32, tag="i100")
            nc.sync.dma_start(i100[:], i100_d[:])

            # ---- PE fences: make PE observe each input-DMA semaphore on a
            # tiny standalone matmul, so real (accumulating) matmuls don't
            # exceed the per-instruction sync-wait limit.
            for fap in (
                x0[:, 0, 0:1], x1[:, 0, 0:1], w1sb[:, 0, 0:1],
                w2sb[:, 0, 0:1], osum[:, 0:1], orep[:, 0:1], i100[:, 0:1],
            ):
                psf = psum.tile([1, 1], F32, tag="psf")
                nc.tensor.matmul(psf[:], fap, fap, start=True, stop=True)

            # ---- stage A: conv1x1  xc[cc, pix'] over the padded grid ----
            xc = mpool.tile([CC, HP, WP], F32, tag="xc")
            xc_flat = xc[:].rearrange("c h w -> c (h w)")
            x0_flat = x0[:].rearrange("c h w -> c (h w)")
            x1_flat = x1[:].rearrange("c h w -> c (h w)")
            CHUNK = 512
            nchunks = (NPADPIX + CHUNK - 1) // CHUNK  # 5 (last = 400)
            for i in range(nchunks):
                n0 = i * CHUNK
                n1 = min(NPADPIX, n0 + CHUNK)
                ps = psum.tile([CC, CHUNK], F32, tag="ps")
                nc.tensor.matmul(
                    ps[:, : n1 - n0], w1sb[:, 0, :], x0_flat[:, n0:n1],
                    start=True, stop=False,
                )
                nc.tensor.matmul(
                    ps[:, : n1 - n0], w1sb[:, 1, :], x1_flat[:, n0:n1],
                    start=False, stop=True,
                )
                # += b1 while copying PSUM -> SBUF
                nc.vector.tensor_scalar_add(
                    xc_flat[:, n0:n1], ps[:, : n1 - n0], b1c[:, 0:1]
                )

            # ---- stage B: conv3x3 -> mask_raw, fused exp((.)+b2) -------
            # output pixels: h in 0..31 (padded row h+2), w in 0..63 (padded col w+2)
            msk_e = mpool.tile([NM, NPIX], F32, tag="msk_e")  # exp(mask_raw)
            HROWS = 8  # rows per 512-chunk
            for i in range(HL // HROWS):  # 4 chunks
                psm = psum.tile([NM, HROWS, W], F32, tag="ps")
                for tap in range(9):
                    dy, dx = tap // 3, tap % 3
                    rhs = xc[:, i * HROWS + 1 + dy : i * HROWS + 1 + dy + HROWS,
                             1 + dx : 1 + dx + W]
                    nc.tensor.matmul(
                        psm[:], w2sb[:, tap, :], rhs,
                        start=(tap == 0), stop=(tap == 8),
                    )
                me = msk_e[:].rearrange("m (h w) -> m h w", w=W)
                nc.scalar.activation(
                    me[:, i * HROWS : (i + 1) * HROWS, :], psm[:],
                    mybir.ActivationFunctionType.Exp, bias=b2c[:, 0:1],
                )

            # ---- stage C: softmax denominators + normalize + transpose --
            # mT[blk][pix, m] = msk_n[m, blk*128 + pix]
            rs = mpool.tile([NQ, NPIX], F32, tag="rs")  # 1/sum per (q, pix)
            msk_n = mpool.tile([NM, NPIX], F32, tag="msk_n")
            mT = mpool.tile([128, NBLK, NM], F32, tag="mT")
            for i in range(NPIX // CHUNK):  # 4
                pss = psum.tile([NQ, CHUNK], F32, tag="ps")
                nc.tensor.matmul(
                    pss[:], osum[:], msk_e[:, i * CHUNK : (i + 1) * CHUNK],
                    start=True, stop=True,
                )
                nc.vector.reciprocal(rs[:, i * CHUNK : (i + 1) * CHUNK], pss[:])
                psr = psum.tile([NM, CHUNK], F32, tag="ps")
                nc.tensor.matmul(
                    psr[:], orep[:], rs[:, i * CHUNK : (i + 1) * CHUNK],
                    start=True, stop=True,
                )
                nc.vector.tensor_mul(
                    msk_n[:, i * CHUNK : (i + 1) * CHUNK],
                    msk_e[:, i * CHUNK : (i + 1) * CHUNK], psr[:],
                )
                # transpose the 4 blocks of this chunk on PE
                for bb in range(4):
                    blk = i * 4 + bb
                    pst = psum_t.tile([128, NM], F32, tag="pst")
                    nc.tensor.matmul(
                        pst[:],
                        msk_n[:, blk * 128 : (blk + 1) * 128],
                        i100[:],
                        start=True, stop=True,
                    )
                    nc.scalar.copy(mT[:, blk, :], pst[:])

            # ---- stage D: combine. 64 chains of 25 MACs ----------------
            # acc_qblk[pix, c] = sum_k mT[pix, blk, q*25+k] * xtw[pix, 2blk+di, dj, c]
            # DVE chains: fused scalar_tensor_tensor (476 ns/tap measured).
            # ACT->Pool chains: scalar.mul produces prod (592 ns), gpsimd
            # tensor_add accumulates (671 ns) - a second parallel pipe.
            # Balance: DVE 11.9 us/chain vs pipe 16.8 -> 38:26 split.
            dve_t = 0.0
            pipe_t = 0.0
            for blk in range(NBLK):
                for q in range(NQ):
                    use_dve = dve_t * 1e-3 <= pipe_t * 1e-3
                    acc = accpool.tile([128, C], F32, tag="acc")
                    if use_dve:
                        dve_t += 25 * 476.0
                        for k in range(KA):
                            di, dj = k // K5, k % K5
                            xin = xtw[:, 2 * blk + di, dj, :]
                            mcol = mT[:, blk, q * KA + k : q * KA + k + 1]
                            if k == 0:
                                nc.vector.tensor_scalar_mul(acc[:], xin, mcol)
                            else:
                                nc.vector.scalar_tensor_tensor(
                                    acc[:], xin, mcol, acc[:],
                                    mybir.AluOpType.mult, mybir.AluOpType.add,
                                )
                    else:
                        pipe_t += 25 * 671.0
                        for k in range(KA):
                            di, dj = k // K5, k % K5
                            xin = xtw[:, 2 * blk + di, dj, :]
                            mcol = mT[:, blk, q * KA + k : q * KA + k + 1]
                            if k == 0:
                                nc.scalar.mul(acc[:], xin, mcol)
                            else:
                                prod = scratch.tile([128, C], BF16, tag="prod")
                                nc.scalar.mul(prod[:], xin, mcol)
                                nc.gpsimd.tensor_add(acc[:], acc[:], prod[:])
                    nc.sync.dma_start(out_d[blk, q], acc[:])

    nc.compile()
    return nc


@lru_cache(maxsize=2)
def _get_program(trace_debug: bool = False):
    return _build_program()


def _host_prep(x, w1, b1, w2, b2):
    """Build per-core input maps."""
    x = np.asarray(x, np.float32)
    w1 = np.asarray(w1, np.float32)
    b1 = np.asarray(b1, np.float32).reshape(CC, 1)
    w2 = np.asarray(w2, np.float32)
    b2 = np.asarray(b2, np.float32).reshape(NM, 1)

    w1t = np.ascontiguousarray(
        w1[:, :, 0, 0].T.reshape(2, 128, CC)
    )  # [c-tile, 128, CC]
    # w2: (100, 64, 3, 3) -> [cc, tap, m]
    w2t = np.ascontiguousarray(w2.transpose(1, 2, 3, 0).reshape(CC, 9, NM))
    osum = np.zeros((NM, NQ), np.float32)
    for q in range(NQ):
        osum[q * KA : (q + 1) * KA, q] = 1.0
    orep = np.ascontiguousarray(osum.T)
    i100 = np.eye(NM, dtype=np.float32)

    in_maps = []
    for s in range(N_CORES):
        b, hh = s // 2, s % 2
        h0 = hh * HL
        xpad = np.zeros((C, HP, WP), np.float32)
        r0 = max(0, h0 - 2)
        r1 = min(H, h0 + HL + 2)
        xpad[:, (r0 - h0 + 2) : (r1 - h0 + 2), 2 : 2 + W] = x[b, :, r0:r1, :]
        # pixel-major transpose + pre-shifted window tiles (bf16)
        xpadT = np.ascontiguousarray(xpad.transpose(1, 2, 0)).astype(_BF16NP)
        # xtw[(rr*64+w), rp, dj, c] = xpadT[rp+rr, w+dj, c]
        xtw = np.empty((2, 64, NRP, K5, C), dtype=_BF16NP)
        for rr in range(2):
            for dj in range(K5):
                # [NRP rows, 64 w, C] -> transpose to [64, NRP, C]
                xtw[rr, :, :, dj, :] = xpadT[
                    rr : rr + NRP, dj : dj + 64, :
                ].transpose(1, 0, 2)
        xtw = np.ascontiguousarray(xtw.reshape(128, NRP, K5, C))
        in_maps.append(
            {
                "x0": np.ascontiguousarray(xpad[:128]),
                "x1": np.ascontiguousarray(xpad[128:]),
                "xtw": xtw,
                "w1t": w1t,
                "w2t": w2t,
                "b1v": b1,
                "b2v": b2,
                "osum": osum,
                "orep": orep,
                "i100": i100,
            }
        )
    return in_maps


def _host_post(results):
    """Reassemble full output from per-core results."""
    out = np.empty((B, C, H * SF, W * SF), np.float32)
    for s in range(N_CORES):
        b, hh = s // 2, s % 2
        o = results[s]["out"]  # [NBLK, NQ, 128, C]
        # [blk, (s1, s2), (rr, w), c] -> [c, (blk, rr, s1), (w, s2)]
        o = o.reshape(NBLK, SF, SF, 2, W, C)
        o = o.transpose(5, 0, 3, 1, 4, 2).reshape(C, HL * SF, W * SF)
        out[b, :, hh * HL * SF : (hh + 1) * HL * SF, :] = o
    return out


def kernel(x, w1, b1, w2, b2):
    nc = _get_program(False)
    in_maps = _host_prep(x, w1, b1, w2, b2)
    res = run_bass_kernel_spmd(nc, in_maps, list(range(N_CORES)))
    return _host_post(res.results)


# revision 6
# speedup vs baseline: 1.2791x; 1.0095x over previous
"""CARAFE upsampling kernel for 8 Trainium2 NeuronCores.

Reference op (per batch b):
  xc   = conv1x1(x, w1) + b1                     # (CC=64, H, W)
  mask = conv3x3(xc, w2, pad=1) + b2             # (100, H, W)
  mask = softmax over the 25 kernel taps (per q in 4 = SF*SF groups)
  out[q, c, h, w] = sum_k mask[q, k, h, w] * x[c, h+di-2, w+dj-2]
  out pixel-shuffled by SF=2 -> (C, 2H, 2W)

Sharding: 8 shards = batch(4) x H-halves(2).

Device-side layout: the mask pipeline (stages A-C) runs in channel-major
layout; the combine runs in PIXEL-major layout (pixels on partitions,
channels on the free dim) so each mask value is a per-partition scalar
and the 25-tap accumulation is a chain of fused scalar_tensor_tensor
MACs split across the DVE and GPSIMD engines. The host ships pre-shifted
bf16 x-window tiles xtw[(rr,w)=128, 35 row-pair starts, 5 dj, 256 c].
"""

import os
from functools import lru_cache

import numpy as np

import concourse.bass as bass
import concourse.mybir as mybir
from concourse import bacc
import concourse.tile as tile
from concourse.bass_utils import run_bass_kernel_spmd

F32 = mybir.dt.float32
BF16 = mybir.dt.bfloat16
import ml_dtypes as _mld

_BF16NP = _mld.bfloat16

# Problem constants (hardcoded; kernel.py must be self-contained).
B, C, H, W = 4, 256, 64, 64
CC = 64           # compressed channels
SF = 2            # scale factor
K5 = 5            # up-kernel
KA = K5 * K5      # 25 taps
NQ = SF * SF      # 4 quadrants
NM = NQ * KA      # 100 mask channels

HL = 32           # local (per-shard) output rows
HP = HL + 4       # padded rows
WP = W + 4        # padded cols
NPIX = HL * W     # 2048 output pixels per shard
NPADPIX = HP * WP # 2448 padded pixels
NBLK = HL // 2    # 16 pixel blocks of 128 = 2 rows x 64 cols
NRP = HL + 3      # 35 row-pair starts (padded rows rp, rp+1 for rp in 0..34)

N_CORES = 8

# Combine-chain engine split: DVE op ~329 ns vs GPSIMD op ~450 ns.
_DVE_NS, _GPS_NS = 329.0, 450.0


def _build_program():
    """Build the SPMD Bass program (identical on all cores)."""
    nc = bacc.Bacc("TRN2", target_bir_lowering=False, debug=False)

    # ---- DRAM parameters -------------------------------------------------
    x0_d = nc.dram_tensor("x0", [128, HP, WP], F32, kind="ExternalInput")
    x1_d = nc.dram_tensor("x1", [128, HP, WP], F32, kind="ExternalInput")
    w1t_d = nc.dram_tensor("w1t", [2, 128, CC], F32, kind="ExternalInput")
    w2t_d = nc.dram_tensor("w2t", [CC, 9, NM], F32, kind="ExternalInput")
    b1_d = nc.dram_tensor("b1v", [CC, 1], F32, kind="ExternalInput")
    b2_d = nc.dram_tensor("b2v", [NM, 1], F32, kind="ExternalInput")
    osum_d = nc.dram_tensor("osum", [NM, NQ], F32, kind="ExternalInput")
    orep_d = nc.dram_tensor("orep", [NQ, NM], F32, kind="ExternalInput")
    i100_d = nc.dram_tensor("i100", [NM, NM], F32, kind="ExternalInput")
    # pre-shifted pixel-major window tiles:
    # xtw[(rr*64+w), rp, dj, c] = xpadT[rp+rr, w+dj, c]  (bf16)
    xtw_d = nc.dram_tensor("xtw", [128, NRP, K5, C], BF16, kind="ExternalInput")

    # out[blk, q, (rr*64+w), c]
    out_d = nc.dram_tensor("out", [NBLK, NQ, 128, C], BF16, kind="ExternalOutput")

    with tile.TileContext(nc) as tc:
        with (
            tc.tile_pool(name="xpool", bufs=1) as xpool,
            tc.tile_pool(name="wpool", bufs=1) as wpool,
            tc.tile_pool(name="mpool", bufs=1) as mpool,
            tc.tile_pool(name="acc", bufs=8) as accpool,
            tc.tile_pool(name="scratch", bufs=4) as scratch,
            tc.tile_pool(name="psum", bufs=2, space="PSUM") as psum,
            tc.tile_pool(name="psum_t", bufs=4, space="PSUM") as psum_t,
        ):
            # ---- load inputs -------------------------------------------
            x0 = xpool.tile([128, HP, WP], F32)
            x1 = xpool.tile([128, HP, WP], F32)
            nc.sync.dma_start(x0[:], x0_d[:])
            nc.sync.dma_start(x1[:], x1_d[:])

            xtw = xpool.tile([128, NRP, K5, C], BF16, tag="xtw")
            # split by rp range so early blocks' windows land first
            RSPLIT = (0, 9, 18, 27, NRP)
            for s in range(4):
                nc.sync.dma_start(
                    xtw[:, RSPLIT[s] : RSPLIT[s + 1]],
                    xtw_d[:, RSPLIT[s] : RSPLIT[s + 1]],
                )

            # partition dim must come first for SBUF: store as [128, 2, CC]
            w1sb = wpool.tile([128, 2, CC], F32, tag="w1sb")
            nc.sync.dma_start(w1sb[:, 0, :], w1t_d[0])
            nc.sync.dma_start(w1sb[:, 1, :], w1t_d[1])

            w2sb = wpool.tile([CC, 9, NM], F32, tag="w2sb")
            nc.sync.dma_start(w2sb[:], w2t_d[:])

            b1c = wpool.tile([CC, 1], F32, tag="b1c")
            nc.sync.dma_start(b1c[:], b1_d[:])
            b2c = wpool.tile([NM, 1], F32, tag="b2c")
            nc.sync.dma_start(b2c[:], b2_d[:])
            osum = wpool.tile([NM, NQ], F32, tag="osum")
            nc.sync.dma_start(osum[:], osum_d[:])
            orep = wpool.tile([NQ, NM], F32, tag="orep")
            nc.sync.dma_start(orep[:], orep_d[:])
            i100 = wpool.tile([NM, NM], F32, tag="i100")
            nc.sync.dma_start(i100[:], i100_d[:])

            # ---- PE fences: make PE observe each input-DMA semaphore on a
            # tiny standalone matmul, so real (accumulating) matmuls don't
            # exceed the per-instruction sync-wait limit.
            for fap in (
                x0[:, 0, 0:1], x1[:, 0, 0:1], w1sb[:, 0, 0:1],
                w2sb[:, 0, 0:1], osum[:, 0:1], orep[:, 0:1], i100[:, 0:1],
            ):
                psf = psum.tile([1, 1], F32, tag="psf")
                nc.tensor.matmul(psf[:], fap, fap, start=True, stop=True)

            # ---- stage A: conv1x1  xc[cc, pix'] over the padded grid ----
            xc = mpool.tile([CC, HP, WP], F32, tag="xc")
            xc_flat = xc[:].rearrange("c h w -> c (h w)")
            x0_flat = x0[:].rearrange("c h w -> c (h w)")
            x1_flat = x1[:].rearrange("c h w -> c (h w)")
            CHUNK = 512
            nchunks = (NPADPIX + CHUNK - 1) // CHUNK  # 5 (last = 400)
            for i in range(nchunks):
                n0 = i * CHUNK
                n1 = min(NPADPIX, n0 + CHUNK)
                ps = psum.tile([CC, CHUNK], F32, tag="ps")
                nc.tensor.matmul(
                    ps[:, : n1 - n0], w1sb[:, 0, :], x0_flat[:, n0:n1],
                    start=True, stop=False,
                )
                nc.tensor.matmul(
                    ps[:, : n1 - n0], w1sb[:, 1, :], x1_flat[:, n0:n1],
                    start=False, stop=True,
                )
                # += b1 while copying PSUM -> SBUF
                nc.vector.tensor_scalar_add(
                    xc_flat[:, n0:n1], ps[:, : n1 - n0], b1c[:, 0:1]
                )

            # ---- stage B: conv3x3 -> mask_raw, fused exp((.)+b2) -------
            # output pixels: h in 0..31 (padded row h+2), w in 0..63 (padded col w+2)
            msk_e = mpool.tile([NM, NPIX], F32, tag="msk_e")  # exp(mask_raw)
            HROWS = 8  # rows per 512-chunk
            for i in range(HL // HROWS):  # 4 chunks
                psm = psum.tile([NM, HROWS, W], F32, tag="ps")
                for tap in range(9):
                    dy, dx = tap // 3, tap % 3
                    rhs = xc[:, i * HROWS + 1 + dy : i * HROWS + 1 + dy + HROWS,
                             1 + dx : 1 + dx + W]
                    nc.tensor.matmul(
                        psm[:], w2sb[:, tap, :], rhs,
                        start=(tap == 0), stop=(tap == 8),
                    )
                me = msk_e[:].rearrange("m (h w) -> m h w", w=W)
                nc.scalar.activation(
                    me[:, i * HROWS : (i + 1) * HROWS, :], psm[:],
                    mybir.ActivationFunctionType.Exp, bias=b2c[:, 0:1],
                )

            # ---- stage C: softmax denominators + normalize + transpose --
            # mT[blk][pix, m] = msk_n[m, blk*128 + pix]
            rs = mpool.tile([NQ, NPIX], F32, tag="rs")  # 1/sum per (q, pix)
            msk_n = mpool.tile([NM, NPIX], F32, tag="msk_n")
            mT = mpool.tile([128, NBLK, NM], F32, tag="mT")
            for i in range(NPIX // CHUNK):  # 4
                pss = psum.tile([NQ, CHUNK], F32, tag="ps")
                nc.tensor.matmul(
                    pss[:], osum[:], msk_e[:, i * CHUNK : (i + 1) * CHUNK],
                    start=True, stop=True,
                )
                nc.vector.reciprocal(rs[:, i * CHUNK : (i + 1) * CHUNK], pss[:])
                psr = psum.tile([NM, CHUNK], F32, tag="ps")
                nc.tensor.matmul(
                    psr[:], orep[:], rs[:, i * CHUNK : (i + 1) * CHUNK],
                    start=True, stop=True,
                )
                nc.vector.tensor_mul(
                    msk_n[:, i * CHUNK : (i + 1) * CHUNK],
                    msk_e[:, i * CHUNK : (i + 1) * CHUNK], psr[:],
                )
                # transpose the 4 blocks of this chunk on PE
                for bb in range(4):
                    blk = i * 4 + bb
                    pst = psum_t.tile([128, NM], F32, tag="pst")
                    nc.tensor.matmul(
                        pst[:],
                        msk_n[:, blk * 128 : (blk + 1) * 128],
                        i100[:],
                        start=True, stop=True,
                    )
                    nc.scalar.copy(mT[:, blk, :], pst[:])

            # ---- stage D: combine. 64 chains of 25 MACs ----------------
            # acc_qblk[pix, c] = sum_k mT[pix, blk, q*25+k] * xtw[pix, 2blk+di, dj, c]
            # DVE chains: fused scalar_tensor_tensor (476 ns/tap measured).
            # ACT->Pool chains: scalar.mul produces prod (592 ns), gpsimd
            # tensor_add accumulates (671 ns) - a second parallel pipe.
            # Balance: DVE 11.9 us/chain vs pipe 16.8 -> 38:26 split.
            dve_t = 0.0
            pipe_t = 0.0
            for blk in range(NBLK):
                for q in range(NQ):
                    use_dve = dve_t * 1e-3 <= pipe_t * 1e-3
                    acc = accpool.tile([128, C], BF16, tag="acc")
                    if use_dve:
                        dve_t += 25 * 476.0
                        for k in range(KA):
                            di, dj = k // K5, k % K5
                            xin = xtw[:, 2 * blk + di, dj, :]
                            mcol = mT[:, blk, q * KA + k : q * KA + k + 1]
                            if k == 0:
                                nc.vector.tensor_scalar_mul(acc[:], xin, mcol)
                            else:
                                nc.vector.scalar_tensor_tensor(
                                    acc[:], xin, mcol, acc[:],
                                    mybir.AluOpType.mult, mybir.AluOpType.add,
                                )
                    else:
                        pipe_t += 25 * 640.0
                        for k in range(KA):
                            di, dj = k // K5, k % K5
                            xin = xtw[:, 2 * blk + di, dj, :]
                            mcol = mT[:, blk, q * KA + k : q * KA + k + 1]
                            if k == 0:
                                nc.scalar.mul(acc[:], xin, mcol)
                            else:
                                prod = scratch.tile([128, C], BF16, tag="prod")
                                nc.scalar.mul(prod[:], xin, mcol)
                                nc.gpsimd.tensor_add(acc[:], acc[:], prod[:])
                    nc.sync.dma_start(out_d[blk, q], acc[:])

    nc.compile()
    return nc


@lru_cache(maxsize=2)
def _get_program(trace_debug: bool = False):
    return _build_program()


def _host_prep(x, w1, b1, w2, b2):
    """Build per-core input maps."""
    x = np.asarray(x, np.float32)
    w1 = np.asarray(w1, np.float32)
    b1 = np.asarray(b1, np.float32).reshape(CC, 1)
    w2 = np.asarray(w2, np.float32)
    b2 = np.asarray(b2, np.float32).reshape(NM, 1)

    w1t = np.ascontiguousarray(
        w1[:, :, 0, 0].T.reshape(2, 128, CC)
    )  # [c-tile, 128, CC]
    # w2: (100, 64, 3, 3) -> [cc, tap, m]
    w2t = np.ascontiguousarray(w2.transpose(1, 2, 3, 0).reshape(CC, 9, NM))
    osum = np.zeros((NM, NQ), np.float32)
    for q in range(NQ):
        osum[q * KA : (q + 1) * KA, q] = 1.0
    orep = np.ascontiguousarray(osum.T)
    i100 = np.eye(NM, dtype=np.float32)

    in_maps = []
    for s in range(N_CORES):
        b, hh = s // 2, s % 2
        h0 = hh * HL
        xpad = np.zeros((C, HP, WP), np.float32)
        r0 = max(0, h0 - 2)
        r1 = min(H, h0 + HL + 2)
        xpad[:, (r0 - h0 + 2) : (r1 - h0 + 2), 2 : 2 + W] = x[b, :, r0:r1, :]
        # pixel-major transpose + pre-shifted window tiles (bf16)
        xpadT = np.ascontiguousarray(xpad.transpose(1, 2, 0)).astype(_BF16NP)
        # xtw[(rr*64+w), rp, dj, c] = xpadT[rp+rr, w+dj, c]
        xtw = np.empty((2, 64, NRP, K5, C), dtype=_BF16NP)
        for rr in range(2):
            for dj in range(K5):
                # [NRP rows, 64 w, C] -> transpose to [64, NRP, C]
                xtw[rr, :, :, dj, :] = xpadT[
                    rr : rr + NRP, dj : dj + 64, :
                ].transpose(1, 0, 2)
        xtw = np.ascontiguousarray(xtw.reshape(128, NRP, K5, C))
        in_maps.append(
            {
                "x0": np.ascontiguousarray(xpad[:128]),
                "x1": np.ascontiguousarray(xpad[128:]),
                "xtw": xtw,
                "w1t": w1t,
                "w2t": w2t,
                "b1v": b1,
                "b2v": b2,
                "osum": osum,
                "orep": orep,
                "i100": i100,
            }
        )
    return in_maps


def _host_post(results):
    """Reassemble full output from per-core results."""
    out = np.empty((B, C, H * SF, W * SF), np.float32)
    for s in range(N_CORES):
        b, hh = s // 2, s % 2
        o = np.asarray(results[s]["out"], np.float32)  # [NBLK, NQ, 128, C]
        # [blk, (s1, s2), (rr, w), c] -> [c, (blk, rr, s1), (w, s2)]
        o = o.reshape(NBLK, SF, SF, 2, W, C)
        o = o.transpose(5, 0, 3, 1, 4, 2).reshape(C, HL * SF, W * SF)
        out[b, :, hh * HL * SF : (hh + 1) * HL * SF, :] = o
    return out


def kernel(x, w1, b1, w2, b2):
    nc = _get_program(False)
    in_maps = _host_prep(x, w1, b1, w2, b2)
    res = run_bass_kernel_spmd(nc, in_maps, list(range(N_CORES)))
    return _host_post(res.results)


# revision 8
# speedup vs baseline: 1.7244x; 1.3482x over previous
"""CARAFE upsampling kernel for 8 Trainium2 NeuronCores.

Reference op (per batch b):
  xc   = conv1x1(x, w1) + b1                     # (CC=64, H, W)
  mask = conv3x3(xc, w2, pad=1) + b2             # (100, H, W)
  mask = softmax over the 25 kernel taps (per q in 4 = SF*SF groups)
  out[q, c, h, w] = sum_k mask[q, k, h, w] * x[c, h+di-2, w+dj-2]
  out pixel-shuffled by SF=2 -> (C, 2H, 2W)

Sharding: 8 shards = batch(4) x H-halves(2).

Combine strategy (channel-major, wide bf16 ops in DVE 2x mode): per
(tap k, quadrant q) the normalized mask row [2048 px] is partition-
broadcast to a [128, 2048] bf16 tile via a stride-0 DRAM-source DMA
(runs on the DMA engines, off the compute path). The 25-tap x 4q x 2ch
accumulation then runs as [128, 2048] ops on balanced lanes:
  - PAIR: DVE tensor_mul + DVE tensor_add (both bf16 2x, ~1.2 us each)
  - XPOOL: DVE tensor_mul -> GPSIMD tensor_add (~4.2 us)
Each (q, ch) keeps one accumulator per adding engine; partials are
merged on DVE at the end and written out in bf16.
"""

import os
from functools import lru_cache

import numpy as np

import concourse.bass as bass
import concourse.mybir as mybir
from concourse import bacc
import concourse.tile as tile
from concourse.bass_utils import run_bass_kernel_spmd

F32 = mybir.dt.float32
BF16 = mybir.dt.bfloat16
import ml_dtypes as _mld

_BF16NP = _mld.bfloat16

B, C, H, W = 4, 256, 64, 64
CC = 64
SF = 2
K5 = 5
KA = K5 * K5
NQ = SF * SF
NM = NQ * KA

HL = 32
HP = HL + 4
WP = W + 4
NPIX = HL * W
NPADPIX = HP * WP

N_CORES = 8

# measured per-[128,2048]-op engine costs (ns) for lane balancing
_DVE_MUL = 1250.0
_DVE_ADD = 1210.0
_POOL_ADD = 4200.0


def _build_program():
    nc = bacc.Bacc("TRN2", target_bir_lowering=False, debug=False)

    x0_d = nc.dram_tensor("x0", [128, HP, WP], F32, kind="ExternalInput")
    x1_d = nc.dram_tensor("x1", [128, HP, WP], F32, kind="ExternalInput")
    xb0_d = nc.dram_tensor("xb0", [128, HP, WP], BF16, kind="ExternalInput")
    xb1_d = nc.dram_tensor("xb1", [128, HP, WP], BF16, kind="ExternalInput")
    w1t_d = nc.dram_tensor("w1t", [2, 128, CC], F32, kind="ExternalInput")
    w2t_d = nc.dram_tensor("w2t", [CC, 9, NM], F32, kind="ExternalInput")
    b1_d = nc.dram_tensor("b1v", [CC, 1], F32, kind="ExternalInput")
    b2_d = nc.dram_tensor("b2v", [NM, 1], F32, kind="ExternalInput")
    osum_d = nc.dram_tensor("osum", [NM, NQ], F32, kind="ExternalInput")
    orep_d = nc.dram_tensor("orep", [NQ, NM], F32, kind="ExternalInput")
    # normalized-mask staging in DRAM for stride-0 broadcast reads
    msk_d = nc.dram_tensor("mskd", [NM, NPIX], BF16, kind="Internal")

    out_d = nc.dram_tensor("out", [2, 128, NQ, NPIX], BF16, kind="ExternalOutput")

    with tile.TileContext(nc) as tc:
        with (
            tc.tile_pool(name="xpool", bufs=1) as xpool,
            tc.tile_pool(name="wpool", bufs=1) as wpool,
            tc.tile_pool(name="mpool", bufs=1) as mpool,
            tc.tile_pool(name="acc", bufs=1) as accpool,
            tc.tile_pool(name="bcast", bufs=6) as bcpool,
            tc.tile_pool(name="prod", bufs=6) as prpool,
            tc.tile_pool(name="psum", bufs=2, space="PSUM") as psum,
        ):
            # ---- load inputs -------------------------------------------
            x0 = xpool.tile([128, HP, WP], F32)
            x1 = xpool.tile([128, HP, WP], F32)
            nc.sync.dma_start(x0[:], x0_d[:])
            nc.sync.dma_start(x1[:], x1_d[:])
            xb0 = xpool.tile([128, HP, WP], BF16, tag="xb0")
            xb1 = xpool.tile([128, HP, WP], BF16, tag="xb1")
            nc.sync.dma_start(xb0[:], xb0_d[:])
            nc.sync.dma_start(xb1[:], xb1_d[:])

            w1sb = wpool.tile([128, 2, CC], F32, tag="w1sb")
            nc.sync.dma_start(w1sb[:, 0, :], w1t_d[0])
            nc.sync.dma_start(w1sb[:, 1, :], w1t_d[1])
            w2sb = wpool.tile([CC, 9, NM], F32, tag="w2sb")
            nc.sync.dma_start(w2sb[:], w2t_d[:])
            b1c = wpool.tile([CC, 1], F32, tag="b1c")
            nc.sync.dma_start(b1c[:], b1_d[:])
            b2c = wpool.tile([NM, 1], F32, tag="b2c")
            nc.sync.dma_start(b2c[:], b2_d[:])
            osum = wpool.tile([NM, NQ], F32, tag="osum")
            nc.sync.dma_start(osum[:], osum_d[:])
            orep = wpool.tile([NQ, NM], F32, tag="orep")
            nc.sync.dma_start(orep[:], orep_d[:])

            # ---- PE fences ---------------------------------------------
            for fap in (
                x0[:, 0, 0:1], x1[:, 0, 0:1], w1sb[:, 0, 0:1],
                w2sb[:, 0, 0:1], osum[:, 0:1], orep[:, 0:1],
            ):
                psf = psum.tile([1, 1], F32, tag="psf")
                nc.tensor.matmul(psf[:], fap, fap, start=True, stop=True)

            # ---- stage A: conv1x1 --------------------------------------
            xc = mpool.tile([CC, HP, WP], F32, tag="xc")
            xc_flat = xc[:].rearrange("c h w -> c (h w)")
            x0_flat = x0[:].rearrange("c h w -> c (h w)")
            x1_flat = x1[:].rearrange("c h w -> c (h w)")
            CHUNK = 512
            nchunks = (NPADPIX + CHUNK - 1) // CHUNK
            for i in range(nchunks):
                n0 = i * CHUNK
                n1 = min(NPADPIX, n0 + CHUNK)
                ps = psum.tile([CC, CHUNK], F32, tag="ps")
                nc.tensor.matmul(
                    ps[:, : n1 - n0], w1sb[:, 0, :], x0_flat[:, n0:n1],
                    start=True, stop=False,
                )
                nc.tensor.matmul(
                    ps[:, : n1 - n0], w1sb[:, 1, :], x1_flat[:, n0:n1],
                    start=False, stop=True,
                )
                nc.vector.tensor_scalar_add(
                    xc_flat[:, n0:n1], ps[:, : n1 - n0], b1c[:, 0:1]
                )

            # ---- stage B: conv3x3 -> exp -------------------------------
            msk_e = mpool.tile([NM, NPIX], F32, tag="msk_e")
            HROWS = 8
            for i in range(HL // HROWS):
                psm = psum.tile([NM, HROWS, W], F32, tag="ps")
                for tap in range(9):
                    dy, dx = tap // 3, tap % 3
                    rhs = xc[:, i * HROWS + 1 + dy : i * HROWS + 1 + dy + HROWS,
                             1 + dx : 1 + dx + W]
                    nc.tensor.matmul(
                        psm[:], w2sb[:, tap, :], rhs,
                        start=(tap == 0), stop=(tap == 8),
                    )
                me = msk_e[:].rearrange("m (h w) -> m h w", w=W)
                nc.scalar.activation(
                    me[:, i * HROWS : (i + 1) * HROWS, :], psm[:],
                    mybir.ActivationFunctionType.Exp, bias=b2c[:, 0:1],
                )

            # ---- stage C: normalize (bf16) + stage to DRAM -------------
            rs = mpool.tile([NQ, NPIX], F32, tag="rs")
            msk_nb = mpool.tile([NM, NPIX], BF16, tag="msk_nb")
            for i in range(NPIX // CHUNK):
                pss = psum.tile([NQ, CHUNK], F32, tag="ps")
                nc.tensor.matmul(
                    pss[:], osum[:], msk_e[:, i * CHUNK : (i + 1) * CHUNK],
                    start=True, stop=True,
                )
                nc.vector.reciprocal(rs[:, i * CHUNK : (i + 1) * CHUNK], pss[:])
                psr = psum.tile([NM, CHUNK], F32, tag="ps")
                nc.tensor.matmul(
                    psr[:], orep[:], rs[:, i * CHUNK : (i + 1) * CHUNK],
                    start=True, stop=True,
                )
                nc.vector.tensor_mul(
                    msk_nb[:, i * CHUNK : (i + 1) * CHUNK],
                    msk_e[:, i * CHUNK : (i + 1) * CHUNK], psr[:],
                )
                nc.sync.dma_start(
                    msk_d[:, i * CHUNK : (i + 1) * CHUNK],
                    msk_nb[:, i * CHUNK : (i + 1) * CHUNK],
                )

            # ---- stage D: combine --------------------------------------
            xbs = (xb0, xb1)
            acc_d = {}
            acc_p = {}
            dve_t = pool_t = 0.0
            for k in range(KA):
                di, dj = k // K5, k % K5
                for q in range(NQ):
                    row = q * KA + k
                    bc = bcpool.tile([128, NPIX], BF16, tag="bc")
                    nc.sync.dma_start(
                        bc[:], msk_d[row : row + 1, :].broadcast_to((128, NPIX))
                    )
                    bcv = bc[:].rearrange("p (h w) -> p h w", w=W)
                    for ch in range(2):
                        key = (q, ch)
                        win = xbs[ch][:, di : di + HL, dj : dj + W]
                        t_pair = dve_t + _DVE_MUL + _DVE_ADD
                        t_pool = max(dve_t + _DVE_MUL, pool_t) + _POOL_ADD
                        if t_pair <= t_pool:
                            # DVE mul + DVE add into acc_d
                            if key not in acc_d:
                                a = accpool.tile(
                                    [128, NPIX], BF16, tag=f"acc_d{q}_{ch}"
                                )
                                acc_d[key] = a
                                av = a[:].rearrange("p (h w) -> p h w", w=W)
                                nc.vector.tensor_mul(av, win, bcv)
                                dve_t += _DVE_MUL
                            else:
                                a = acc_d[key]
                                pr = prpool.tile([128, NPIX], BF16, tag="prd")
                                prv = pr[:].rearrange("p (h w) -> p h w", w=W)
                                nc.vector.tensor_mul(prv, win, bcv)
                                nc.vector.tensor_add(a[:], a[:], pr[:])
                                dve_t += _DVE_MUL + _DVE_ADD
                        else:
                            # DVE mul -> Pool add into acc_p
                            if key not in acc_p:
                                a = accpool.tile(
                                    [128, NPIX], BF16, tag=f"acc_p{q}_{ch}"
                                )
                                acc_p[key] = a
                                av = a[:].rearrange("p (h w) -> p h w", w=W)
                                nc.vector.tensor_mul(av, win, bcv)
                                dve_t += _DVE_MUL
                            else:
                                a = acc_p[key]
                                pr = prpool.tile([128, NPIX], BF16, tag="prp")
                                prv = pr[:].rearrange("p (h w) -> p h w", w=W)
                                nc.vector.tensor_mul(prv, win, bcv)
                                nc.gpsimd.tensor_add(a[:], a[:], pr[:])
                                dve_t += _DVE_MUL
                                pool_t = max(pool_t, dve_t) + _POOL_ADD

            # ---- merge partials + write out ----------------------------
            for q in range(NQ):
                for ch in range(2):
                    a = acc_d[(q, ch)]
                    if (q, ch) in acc_p:
                        nc.vector.tensor_add(a[:], a[:], acc_p[(q, ch)][:])
                    nc.sync.dma_start(out_d[ch, :, q, :], a[:])

    nc.compile()
    return nc


@lru_cache(maxsize=2)
def _get_program(trace_debug: bool = False):
    return _build_program()


def _host_prep(x, w1, b1, w2, b2):
    x = np.asarray(x, np.float32)
    w1 = np.asarray(w1, np.float32)
    b1 = np.asarray(b1, np.float32).reshape(CC, 1)
    w2 = np.asarray(w2, np.float32)
    b2 = np.asarray(b2, np.float32).reshape(NM, 1)

    w1t = np.ascontiguousarray(w1[:, :, 0, 0].T.reshape(2, 128, CC))
    w2t = np.ascontiguousarray(w2.transpose(1, 2, 3, 0).reshape(CC, 9, NM))
    osum = np.zeros((NM, NQ), np.float32)
    for q in range(NQ):
        osum[q * KA : (q + 1) * KA, q] = 1.0
    orep = np.ascontiguousarray(osum.T)

    in_maps = []
    for s in range(N_CORES):
        b, hh = s // 2, s % 2
        h0 = hh * HL
        xpad = np.zeros((C, HP, WP), np.float32)
        r0 = max(0, h0 - 2)
        r1 = min(H, h0 + HL + 2)
        xpad[:, (r0 - h0 + 2) : (r1 - h0 + 2), 2 : 2 + W] = x[b, :, r0:r1, :]
        xb = xpad.astype(_BF16NP)
        in_maps.append(
            {
                "x0": np.ascontiguousarray(xpad[:128]),
                "x1": np.ascontiguousarray(xpad[128:]),
                "xb0": np.ascontiguousarray(xb[:128]),
                "xb1": np.ascontiguousarray(xb[128:]),
                "w1t": w1t,
                "w2t": w2t,
                "b1v": b1,
                "b2v": b2,
                "osum": osum,
                "orep": orep,
            }
        )
    return in_maps


def _host_post(results):
    out = np.empty((B, C, H * SF, W * SF), np.float32)
    for s in range(N_CORES):
        b, hh = s // 2, s % 2
        o = np.asarray(results[s]["out"], np.float32)  # [2, 128, NQ, NPIX]
        o = o.reshape(2, 128, NQ, HL, W).reshape(C, SF, SF, HL, W)
        o = o.transpose(0, 3, 1, 4, 2).reshape(C, HL * SF, W * SF)
        out[b, :, hh * HL * SF : (hh + 1) * HL * SF, :] = o
    return out


def kernel(x, w1, b1, w2, b2):
    nc = _get_program(False)
    in_maps = _host_prep(x, w1, b1, w2, b2)
    res = run_bass_kernel_spmd(nc, in_maps, list(range(N_CORES)))
    return _host_post(res.results)
